# revision 31
# baseline (speedup 1.0000x reference)
"""MoE kernel for Trainium2 (8 NeuronCores, expert-parallel).

Strategy
--------
N=8192 tokens, D=1024, E=8 experts, DFF=4096, top_k=2. E == n_cores, so
core c owns expert c. The reference computes every expert densely and
masks; mathematically only each token's top-2 experts contribute, so we
dispatch each token to its 2 experts and run the expert MLPs on just the
routed tokens: 4x fewer FLOPs than dense.

Host (sharding/dispatch): gate matmul + top-2 + renormalized combine
weights (0.003%% of total FLOPs), gather each expert's tokens into a
[C, D] batch (C = max expert load, exact - tokens only ever stream
along the matmul free axis, so no padding granularity is needed).
Device (per core): ytT[d, t] = (silu(xgT.T @ w1) @ w2).T for its batch.
Both expert weight matrices are SBUF-resident (bf16: 64+64
KB/partition) and all token blocks stream through them, so HBM traffic
is ~30 MB/core against ~460 us of PE work: compute-bound at the bf16
matmul roofline.
Host (unshard): y[token] = g1 * yt[e1][:, slot1] + g2 * yt[e2][:, slot2]
(each token appears exactly once in each of its two experts' batches;
the renormalized gate weights are applied on host).

Compute dtype knob: "bf16" (fast, ~4e-3 rel err), "f32r" (float32r
matmuls: full fp32 storage, fast PE path but 2x DMA/SBUF), "f32"
(exact, 4x slower PE).
"""

import numpy as np

import concourse.bass as bass
import concourse.bacc as bacc
import concourse.tile as tile
from concourse import mybir
from concourse.bass_utils import run_bass_kernel_spmd

N, D, E, DFF = 8192, 1024, 8, 4096
P = 128

CDT = "bf16"  # "bf16" | "f32r" | "f32"
TRACE = False
LAST_RESULT = None


def _blocks_for(C):
    # Remainder block LAST: small matmuls run efficiently only once the
    # engine queue is saturated and the clock is warm.
    out = [512] * (C // 512)
    if C % 512:
        out.append(C % 512)
    return out


def build_nc(C, cdt):
    """Per-core program: yt[D, C] = (silu(xgT.T @ w1) @ w2).T.

    Weights are SBUF-resident for the whole kernel; token blocks of 512
    stream through. Per block: phase A computes hT = silu(w1-chunks.T @
    xg) into SBUF (f on partitions), phase B accumulates yt d-chunks
    (d on partitions, tokens streaming) over the 32 f-chunks against
    resident w2.
    """
    import ml_dtypes

    if cdt == "bf16":
        sdt, ndt = mybir.dt.bfloat16, ml_dtypes.bfloat16
    elif cdt == "f32r":
        sdt, ndt = mybir.dt.float32r, np.float32
    else:
        sdt, ndt = mybir.dt.float32, np.float32

    nc = bacc.Bacc()
    xgt = nc.dram_tensor("xgt", [D, C], sdt, kind="ExternalInput")
    # Block 0's x plus w1's first 256 f-cols, host-prepacked into ONE
    # contiguous tensor so the whole critical head prefix is a single
    # full-rate DMA (12KB contiguous per partition line).
    B0 = _blocks_for(C)[0]
    xg0p = nc.dram_tensor(
        "xg0p", [P, (D // P) * (B0 + 256)], sdt, kind="ExternalInput"
    )
    w1 = nc.dram_tensor("w1", [D, DFF], sdt, kind="ExternalInput")
    w2 = nc.dram_tensor("w2", [DFF, D], sdt, kind="ExternalInput")
    yt = nc.dram_tensor("yt", [D, C], mybir.dt.float32, kind="ExternalOutput")

    xgt_r = xgt.rearrange("(k p) c -> p k c", p=P)  # [128, 8, C]
    w1_r = w1.rearrange("(k p) f -> p k f", p=P)  # [128, 8, DFF]
    w2_r = w2.rearrange("(kf p) d -> p kf d", p=P)  # [128, 32, D]
    yt_r = yt.rearrange("(dc p) c -> dc p c", p=P)  # [8, 128, C]

    KD = D // P  # 8 k-chunks, first matmul; also 8 output d-chunks
    KF = DFF // P  # 32 k-chunks, second matmul
    blocks = _blocks_for(C)
    f32 = mybir.dt.float32
    ACT = mybir.ActivationFunctionType

    with tile.TileContext(nc) as tc:
        with (
            tc.tile_pool(name="singles", bufs=1) as singles,
            tc.tile_pool(name="xg", bufs=2) as xg_pool,
            tc.tile_pool(name="ht", bufs=1) as h_pool,
            tc.tile_pool(name="yout", bufs=4) as y_pool,
            tc.tile_pool(name="hps", bufs=2, space="PSUM") as hpsum,
            tc.tile_pool(name="yps", bufs=6, space="PSUM") as ypsum,
        ):
            # PE clock warmup: the HAM clock gate holds the PE at 1.2
            # GHz until it has been busy ~3.4us; idle >3.4us re-throttles.
            # Sized to end right as block 0's inputs land (~15us): ~32
            # run cold (107ns), the rest warm (56ns).
            warm = singles.tile([P, P], sdt, name="warm")
            nc.vector.memset(warm, 0.0)
            wps = hpsum.tile([P, P], f32, tag="hps", name="wps")
            NWARM = 60
            for i in range(NWARM):
                nc.tensor.matmul(
                    wps[:, :],
                    lhsT=warm[:, :],
                    rhs=warm[:, :],
                    start=(i == 0),
                    stop=(i == NWARM - 1),
                )

            # DMA program order is FIFO per queue: critical prefix first.
            # Matmul group 0 needs the whole xg block (prepacked, full
            # rate) plus w1[:, :, 0:256]; later w1 chunks stay one group
            # ahead of phase A consumption (128 cols per ~1.7us).
            xg_first = xg_pool.tile([P, KD, B0 + 256], sdt, tag="xg")
            # Four kd-quarters: the kd-0/1 matmuls unblock after the
            # first ~0.4MB (per-slice deps), well before the full prefix.
            xg0p_r = xg0p.rearrange("p (k t) -> p k t", k=KD)
            for k0 in range(0, KD, 2):
                nc.sync.dma_start(
                    out=xg_first[:, k0 : k0 + 2, :], in_=xg0p_r[:, k0 : k0 + 2, :]
                )
            w1_sb = singles.tile([P, KD, DFF], sdt, name="w1_sb")
            for f0, f1 in (
                (256, 768),
                (768, 1280),
                (1280, 2304),
                (2304, 3328),
                (3328, DFF),
                (0, 256),  # blocks 1+ read these; block 0 uses xg0p's copy
            ):
                nc.sync.dma_start(out=w1_sb[:, :, f0:f1], in_=w1_r[:, :, f0:f1])
            w2_sb = singles.tile([P, KF, D], sdt, name="w2_sb")
            W2CH = 4
            for ch in range(W2CH):
                k0, k1 = ch * (KF // W2CH), (ch + 1) * (KF // W2CH)
                nc.sync.dma_start(out=w2_sb[:, k0:k1, :], in_=w2_r[:, k0:k1, :])

            tok0 = 0
            for bi, B in enumerate(blocks):
                if bi == 0:
                    xg_t = xg_first
                else:
                    xg_t = xg_pool.tile([P, KD, B], sdt, tag="xg")
                    nc.sync.dma_start(out=xg_t, in_=xgt_r[:, :, tok0 : tok0 + B])

                hT = h_pool.tile([P, KF, B], sdt, tag="ht")
                # phase A: hT[f, t] = silu(sum_k w1[k, f] * x[k, t])
                for mf in range(KF):
                    ph = hpsum.tile([P, B], f32, tag="hps")
                    for kd in range(KD):
                        if bi == 0 and mf < 2:
                            lhs = xg_t[:, kd, B + mf * P : B + (mf + 1) * P]
                        else:
                            lhs = w1_sb[:, kd, mf * P : (mf + 1) * P]
                        nc.tensor.matmul(
                            ph[:, :],
                            lhsT=lhs,
                            rhs=xg_t[:, kd, 0:B],
                            start=(kd == 0),
                            stop=(kd == KD - 1),
                        )
                    nc.scalar.activation(hT[:, mf, :], ph[:, :], ACT.Silu)

                # phase B: yt[d, t] = sum_f w2[f, d] * hT[f, t]
                # (w2 chunk stationary, tokens streaming - B needs no
                # padding granularity)
                for dc in range(KD):
                    yp = ypsum.tile([P, B], f32, tag="yps", name="yp")
                    for kf in range(KF):
                        nc.tensor.matmul(
                            yp[:, :],
                            lhsT=w2_sb[:, kf, dc * P : (dc + 1) * P],
                            rhs=hT[:, kf, :],
                            start=(kf == 0),
                            stop=(kf == KF - 1),
                        )
                    y_sb = y_pool.tile([P, B], f32, tag="yout")
                    nc.scalar.activation(y_sb[:, :], yp[:, :], ACT.Copy)
                    nc.sync.dma_start(
                        out=yt_r[dc, :, tok0 : tok0 + B], in_=y_sb[:, :]
                    )
                tok0 += B

    if not nc.is_finalized():
        nc.finalize()
    return nc, ndt


def kernel(x, gate_w, w1, w2, top_k):
    global LAST_RESULT
    x = np.asarray(x, dtype=np.float32)
    gate_w = np.asarray(gate_w, dtype=np.float32)
    w1 = np.asarray(w1, dtype=np.float32)
    w2 = np.asarray(w2, dtype=np.float32)
    assert int(top_k) == 2

    n = x.shape[0]
    ar = np.arange(n)

    # --- host routing (matches reference: softmax -> top2 -> renorm) ---
    logits = (x @ gate_w).astype(np.float64)
    i1 = np.argmax(logits, axis=1)
    lm = logits.copy()
    lm[ar, i1] = -np.inf
    i2 = np.argmax(lm, axis=1)
    m1 = logits[ar, i1]
    m2 = logits[ar, i2]
    g1 = 1.0 / (1.0 + np.exp(m2 - m1))  # = p1/(p1+p2)
    g2 = 1.0 - g1

    sel = np.zeros((n, E), dtype=bool)
    sel[ar, i1] = True
    sel[ar, i2] = True

    idxs = [np.nonzero(sel[:, e])[0] for e in range(E)]
    counts = np.array([len(ix) for ix in idxs])
    C = max(int(counts.max()), 512)

    slot_of = np.zeros((n, E), dtype=np.int64)
    for e in range(E):
        slot_of[idxs[e], e] = np.arange(len(idxs[e]))

    nc, ndt = build_nc(C, CDT)

    def prep(a):
        a = np.ascontiguousarray(a).astype(ndt)
        if CDT == "f32r":
            # replicate walrus fp32_to_fp32r: round mantissa to 11 bits
            u = a.view(np.uint32).astype(np.uint64)
            u = (u + 0x800) & 0xFFFFF000
            a = u.astype(np.uint32).view(np.float32)
        return a

    B0 = _blocks_for(C)[0]
    in_maps = []
    for e in range(E):
        ix = idxs[e]
        xg = np.zeros((C, D), dtype=np.float32)
        xg[: len(ix)] = x[ix]
        xgt_e = prep(xg.T)
        w1_e = prep(w1[e])
        # xg0p[p, k, 0:B0] = x block 0; xg0p[p, k, B0:] = w1[k*128+p, 0:256]
        xpart = xgt_e[:, :B0].reshape(D // P, P, B0)
        wpart = w1_e[:, 0:256].reshape(D // P, P, 256)
        xg0p = np.ascontiguousarray(
            np.concatenate([xpart, wpart], axis=2)
            .transpose(1, 0, 2)
            .reshape(P, -1)
        )
        in_maps.append(
            {
                "xgt": xgt_e,
                "xg0p": xg0p,
                "w1": w1_e,
                "w2": prep(w2[e]),
            }
        )

    res = run_bass_kernel_spmd(nc, in_maps, list(range(E)), trace=TRACE)
    LAST_RESULT = res

    outs = np.stack([res.results[e]["yt"] for e in range(E)])  # [E, D, C]
    y = g1[:, None] * outs[i1, :, slot_of[ar, i1]] + g2[:, None] * outs[
        i2, :, slot_of[ar, i2]
    ]
    return y.astype(np.float32)


# revision 32
# speedup vs baseline: 1.0020x; 1.0020x over previous
"""MoE kernel for Trainium2 (8 NeuronCores, expert-parallel).

Strategy
--------
N=8192 tokens, D=1024, E=8 experts, DFF=4096, top_k=2. E == n_cores, so
core c owns expert c. The reference computes every expert densely and
masks; mathematically only each token's top-2 experts contribute, so we
dispatch each token to its 2 experts and run the expert MLPs on just the
routed tokens: 4x fewer FLOPs than dense.

Host (sharding/dispatch): gate matmul + top-2 + renormalized combine
weights (0.003%% of total FLOPs), gather each expert's tokens into a
[C, D] batch (C = max expert load, exact - tokens only ever stream
along the matmul free axis, so no padding granularity is needed).
Device (per core): ytT[d, t] = (silu(xgT.T @ w1) @ w2).T for its batch.
Both expert weight matrices are SBUF-resident (bf16: 64+64
KB/partition) and all token blocks stream through them, so HBM traffic
is ~30 MB/core against ~460 us of PE work: compute-bound at the bf16
matmul roofline.
Host (unshard): y[token] = g1 * yt[e1][:, slot1] + g2 * yt[e2][:, slot2]
(each token appears exactly once in each of its two experts' batches;
the renormalized gate weights are applied on host).

Compute dtype knob: "bf16" (fast, ~4e-3 rel err), "f32r" (float32r
matmuls: full fp32 storage, fast PE path but 2x DMA/SBUF), "f32"
(exact, 4x slower PE).
"""

import numpy as np

import concourse.bass as bass
import concourse.bacc as bacc
import concourse.tile as tile
from concourse import mybir
from concourse.bass_utils import run_bass_kernel_spmd

N, D, E, DFF = 8192, 1024, 8, 4096
P = 128

CDT = "bf16"  # "bf16" | "f32r" | "f32"
TRACE = False
LAST_RESULT = None


def _blocks_for(C):
    # Remainder block LAST: small matmuls run efficiently only once the
    # engine queue is saturated and the clock is warm.
    out = [512] * (C // 512)
    if C % 512:
        out.append(C % 512)
    return out


def build_nc(C, cdt):
    """Per-core program: yt[D, C] = (silu(xgT.T @ w1) @ w2).T.

    Weights are SBUF-resident for the whole kernel; token blocks of 512
    stream through. Per block: phase A computes hT = silu(w1-chunks.T @
    xg) into SBUF (f on partitions), phase B accumulates yt d-chunks
    (d on partitions, tokens streaming) over the 32 f-chunks against
    resident w2.
    """
    import ml_dtypes

    if cdt == "bf16":
        sdt, ndt = mybir.dt.bfloat16, ml_dtypes.bfloat16
    elif cdt == "f32r":
        sdt, ndt = mybir.dt.float32r, np.float32
    else:
        sdt, ndt = mybir.dt.float32, np.float32

    nc = bacc.Bacc()
    xgt = nc.dram_tensor("xgt", [D, C], sdt, kind="ExternalInput")
    # Block 0's x plus w1's first 256 f-cols, host-prepacked into ONE
    # contiguous tensor so the whole critical head prefix is a single
    # full-rate DMA (12KB contiguous per partition line).
    B0 = _blocks_for(C)[0]
    xg0p = nc.dram_tensor(
        "xg0p", [P, (D // P) * (B0 + 256)], sdt, kind="ExternalInput"
    )
    w1 = nc.dram_tensor("w1", [D, DFF], sdt, kind="ExternalInput")
    w2 = nc.dram_tensor("w2", [DFF, D], sdt, kind="ExternalInput")
    yt = nc.dram_tensor("yt", [D, C], mybir.dt.float32, kind="ExternalOutput")

    xgt_r = xgt.rearrange("(k p) c -> p k c", p=P)  # [128, 8, C]
    w1_r = w1.rearrange("(k p) f -> p k f", p=P)  # [128, 8, DFF]
    w2_r = w2.rearrange("(kf p) d -> p kf d", p=P)  # [128, 32, D]
    yt_r = yt.rearrange("(dc p) c -> dc p c", p=P)  # [8, 128, C]

    KD = D // P  # 8 k-chunks, first matmul; also 8 output d-chunks
    KF = DFF // P  # 32 k-chunks, second matmul
    blocks = _blocks_for(C)
    f32 = mybir.dt.float32
    ACT = mybir.ActivationFunctionType

    with tile.TileContext(nc) as tc:
        with (
            tc.tile_pool(name="singles", bufs=1) as singles,
            tc.tile_pool(name="xg", bufs=2) as xg_pool,
            tc.tile_pool(name="ht", bufs=1) as h_pool,
            tc.tile_pool(name="yout", bufs=4) as y_pool,
            tc.tile_pool(name="hps", bufs=2, space="PSUM") as hpsum,
            tc.tile_pool(name="yps", bufs=6, space="PSUM") as ypsum,
        ):
            # PE clock warmup: the HAM clock gate holds the PE at 1.2
            # GHz until it has been busy ~3.4us; idle >3.4us re-throttles.
            # Sized to end right as block 0's inputs land (~15us): ~32
            # run cold (107ns), the rest warm (56ns).
            warm = singles.tile([P, P], sdt, name="warm")
            nc.vector.memset(warm, 0.0)
            wps = hpsum.tile([P, P], f32, tag="hps", name="wps")
            NWARM = 70
            for i in range(NWARM):
                nc.tensor.matmul(
                    wps[:, :],
                    lhsT=warm[:, :],
                    rhs=warm[:, :],
                    start=(i == 0),
                    stop=(i == NWARM - 1),
                )

            # DMA program order is FIFO per queue: critical prefix first.
            # Matmul group 0 needs the whole xg block (prepacked, full
            # rate) plus w1[:, :, 0:256]; later w1 chunks stay one group
            # ahead of phase A consumption (128 cols per ~1.7us).
            xg_first = xg_pool.tile([P, KD, B0 + 256], sdt, tag="xg")
            # Two kd-halves: matmuls on kd 0-3 unblock after the first
            # half (per-slice deps), ~2us before the full prefix lands.
            xg0p_r = xg0p.rearrange("p (k t) -> p k t", k=KD)
            nc.sync.dma_start(out=xg_first[:, 0:4, :], in_=xg0p_r[:, 0:4, :])
            nc.sync.dma_start(out=xg_first[:, 4:8, :], in_=xg0p_r[:, 4:8, :])
            w1_sb = singles.tile([P, KD, DFF], sdt, name="w1_sb")
            for f0, f1 in (
                (256, 768),
                (768, 1280),
                (1280, 2304),
                (2304, 3328),
                (3328, DFF),
                (0, 256),  # blocks 1+ read these; block 0 uses xg0p's copy
            ):
                nc.sync.dma_start(out=w1_sb[:, :, f0:f1], in_=w1_r[:, :, f0:f1])
            w2_sb = singles.tile([P, KF, D], sdt, name="w2_sb")
            W2CH = 4
            for ch in range(W2CH):
                k0, k1 = ch * (KF // W2CH), (ch + 1) * (KF // W2CH)
                nc.sync.dma_start(out=w2_sb[:, k0:k1, :], in_=w2_r[:, k0:k1, :])

            tok0 = 0
            for bi, B in enumerate(blocks):
                if bi == 0:
                    xg_t = xg_first
                else:
                    xg_t = xg_pool.tile([P, KD, B], sdt, tag="xg")
                    nc.sync.dma_start(out=xg_t, in_=xgt_r[:, :, tok0 : tok0 + B])

                hT = h_pool.tile([P, KF, B], sdt, tag="ht")
                # phase A: hT[f, t] = silu(sum_k w1[k, f] * x[k, t])
                for mf in range(KF):
                    ph = hpsum.tile([P, B], f32, tag="hps")
                    for kd in range(KD):
                        if bi == 0 and mf < 2:
                            lhs = xg_t[:, kd, B + mf * P : B + (mf + 1) * P]
                        else:
                            lhs = w1_sb[:, kd, mf * P : (mf + 1) * P]
                        nc.tensor.matmul(
                            ph[:, :],
                            lhsT=lhs,
                            rhs=xg_t[:, kd, 0:B],
                            start=(kd == 0),
                            stop=(kd == KD - 1),
                        )
                    nc.scalar.activation(hT[:, mf, :], ph[:, :], ACT.Silu)

                # phase B: yt[d, t] = sum_f w2[f, d] * hT[f, t]
                # (w2 chunk stationary, tokens streaming - B needs no
                # padding granularity)
                for dc in range(KD):
                    yp = ypsum.tile([P, B], f32, tag="yps", name="yp")
                    for kf in range(KF):
                        nc.tensor.matmul(
                            yp[:, :],
                            lhsT=w2_sb[:, kf, dc * P : (dc + 1) * P],
                            rhs=hT[:, kf, :],
                            start=(kf == 0),
                            stop=(kf == KF - 1),
                        )
                    y_sb = y_pool.tile([P, B], f32, tag="yout")
                    nc.scalar.activation(y_sb[:, :], yp[:, :], ACT.Copy)
                    nc.sync.dma_start(
                        out=yt_r[dc, :, tok0 : tok0 + B], in_=y_sb[:, :]
                    )
                tok0 += B

    if not nc.is_finalized():
        nc.finalize()
    return nc, ndt


def kernel(x, gate_w, w1, w2, top_k):
    global LAST_RESULT
    x = np.asarray(x, dtype=np.float32)
    gate_w = np.asarray(gate_w, dtype=np.float32)
    w1 = np.asarray(w1, dtype=np.float32)
    w2 = np.asarray(w2, dtype=np.float32)
    assert int(top_k) == 2

    n = x.shape[0]
    ar = np.arange(n)

    # --- host routing (matches reference: softmax -> top2 -> renorm) ---
    logits = (x @ gate_w).astype(np.float64)
    i1 = np.argmax(logits, axis=1)
    lm = logits.copy()
    lm[ar, i1] = -np.inf
    i2 = np.argmax(lm, axis=1)
    m1 = logits[ar, i1]
    m2 = logits[ar, i2]
    g1 = 1.0 / (1.0 + np.exp(m2 - m1))  # = p1/(p1+p2)
    g2 = 1.0 - g1

    sel = np.zeros((n, E), dtype=bool)
    sel[ar, i1] = True
    sel[ar, i2] = True

    idxs = [np.nonzero(sel[:, e])[0] for e in range(E)]
    counts = np.array([len(ix) for ix in idxs])
    C = max(int(counts.max()), 512)

    slot_of = np.zeros((n, E), dtype=np.int64)
    for e in range(E):
        slot_of[idxs[e], e] = np.arange(len(idxs[e]))

    nc, ndt = build_nc(C, CDT)

    def prep(a):
        a = np.ascontiguousarray(a).astype(ndt)
        if CDT == "f32r":
            # replicate walrus fp32_to_fp32r: round mantissa to 11 bits
            u = a.view(np.uint32).astype(np.uint64)
            u = (u + 0x800) & 0xFFFFF000
            a = u.astype(np.uint32).view(np.float32)
        return a

    B0 = _blocks_for(C)[0]
    in_maps = []
    for e in range(E):
        ix = idxs[e]
        xg = np.zeros((C, D), dtype=np.float32)
        xg[: len(ix)] = x[ix]
        xgt_e = prep(xg.T)
        w1_e = prep(w1[e])
        # xg0p[p, k, 0:B0] = x block 0; xg0p[p, k, B0:] = w1[k*128+p, 0:256]
        xpart = xgt_e[:, :B0].reshape(D // P, P, B0)
        wpart = w1_e[:, 0:256].reshape(D // P, P, 256)
        xg0p = np.ascontiguousarray(
            np.concatenate([xpart, wpart], axis=2)
            .transpose(1, 0, 2)
            .reshape(P, -1)
        )
        in_maps.append(
            {
                "xgt": xgt_e,
                "xg0p": xg0p,
                "w1": w1_e,
                "w2": prep(w2[e]),
            }
        )

    res = run_bass_kernel_spmd(nc, in_maps, list(range(E)), trace=TRACE)
    LAST_RESULT = res

    outs = np.stack([res.results[e]["yt"] for e in range(E)])  # [E, D, C]
    y = g1[:, None] * outs[i1, :, slot_of[ar, i1]] + g2[:, None] * outs[
        i2, :, slot_of[ar, i2]
    ]
    return y.astype(np.float32)
